# revision 1
# baseline (speedup 1.0000x reference)
"""Trainium2 Bass kernel for LogicGatedSNN.

Math:
  w = ternarize(synapse_states)            # {-1,0,1}, exact in bf16
  current = spike_input @ w.T              # bf16 matmul, fp32 PSUM accum -> exact
  spikes[b,o] = (current[b,o] - T[o] >= 0)
where T[o] folds threshold/membrane/refractory:
  non-refractory: T = thr - DECAY*vmem
  refractory:     T = +-1e30 depending on sign of (DECAY*vmem - thr)
The -T[o] bias is added as one extra K=1 fp32 matmul into the same PSUM
accumulation group, so the epilogue is a single tensor_scalar is_ge.

Sharding: 8 cores = 2 (batch) x 4 (out_features). Per core:
  spike shard [2048, 4096], synapse shard [1024, 4096].
On-chip dataflow per core:
  - synapse: fp32 DMA in -> DVE ternarize (is_gt / is_lt / sub) -> bf16
    -> xbar DMA-transpose into W[128, 32, OS] (W[p, m, o] = w[o, 128m+p])
  - spike: SWDGE cast-DMA (fp32->bf16) -> xbar transpose S[128, 32, 128]
  - matmul: psum[128b, 512o] accumulates 32 chunks (K=128 each) + bias mm
  - DVE is_ge vs 0 -> fp32 out tile -> DMA out
"""

import sys

if "/opt/trn_rl_repo" not in sys.path:
    sys.path.insert(0, "/opt/trn_rl_repo")

import numpy as np

B, IN, OUT = 4096, 4096, 4096
GB, GO = 2, 4  # core grid: batch x out_features
DECAY = 0.8
_TENSORS = {}


def build_core_program(nc, tc, bs, os_, in_, instance=0):
    """Emit the per-core program. bs/os_/in_ = per-core shard dims."""
    import concourse.mybir as mybir
    from concourse.bass import ts

    FP32 = mybir.dt.float32
    BF16 = mybir.dt.bfloat16
    Op = mybir.AluOpType

    if instance == 0:
        spike = nc.dram_tensor("spike", [bs, in_], FP32, kind="ExternalInput")
        syn = nc.dram_tensor("syn", [os_, in_], FP32, kind="ExternalInput")
        thr = nc.dram_tensor("thr", [1, os_], FP32, kind="ExternalInput")
        vmem = nc.dram_tensor("vmem", [1, os_], FP32, kind="ExternalInput")
        refrac = nc.dram_tensor("refrac", [1, os_], FP32, kind="ExternalInput")
        # spikes are 0/1 -> exact in bf16; host casts back to fp32
        out = nc.dram_tensor("spikes", [bs, os_], BF16, kind="ExternalOutput")
        _TENSORS.update(
            spike=spike, syn=syn, thr=thr, vmem=vmem, refrac=refrac, out=out
        )
    else:
        spike, syn, thr, vmem, refrac, out = (
            _TENSORS["spike"],
            _TENSORS["syn"],
            _TENSORS["thr"],
            _TENSORS["vmem"],
            _TENSORS["refrac"],
            _TENSORS["out"],
        )

    KC = in_ // 128  # contraction chunks
    NB = bs // 128  # batch tiles
    NT = 512  # matmul free dim per o-tile
    NO = os_ // NT  # o-tiles

    with (
        tc.tile_pool(name="wpool", bufs=1) as wpool,
        tc.tile_pool(name="synpool", bufs=2) as synpool,
        tc.tile_pool(name="ternpool", bufs=2) as ternpool,
        tc.tile_pool(name="sppool", bufs=2) as sppool,
        tc.tile_pool(name="spool", bufs=3) as spool,
        tc.tile_pool(name="outpool", bufs=4) as outpool,
        tc.tile_pool(name="miscpool", bufs=1) as miscpool,
        tc.tile_pool(name="pspool", bufs=4, space="PSUM") as pspool,
    ):
        # ---- threshold vector negT[0, o] = -(effective threshold) ----
        tv = miscpool.tile([1, os_], FP32, tag="tv")
        vv = miscpool.tile([1, os_], FP32, tag="vv")
        rv = miscpool.tile([1, os_], FP32, tag="rv")
        nc.sync.dma_start(tv[:], thr[:, :])
        nc.sync.dma_start(vv[:], vmem[:, :])
        nc.sync.dma_start(rv[:], refrac[:, :])
        c0 = miscpool.tile([1, os_], FP32, tag="c0")
        nc.vector.tensor_scalar(c0[:], vv[:], DECAY, None, Op.mult)
        nc.vector.tensor_tensor(c0[:], c0[:], tv[:], Op.subtract)  # decay*v - thr
        big = miscpool.tile([1, os_], FP32, tag="big")
        nc.vector.tensor_scalar(big[:], c0[:], 0.0, None, Op.is_ge)
        nc.vector.tensor_scalar(big[:], big[:], 2e30, -1e30, Op.mult, Op.add)
        r01 = miscpool.tile([1, os_], FP32, tag="r01")
        nc.vector.tensor_scalar(r01[:], rv[:], 0.0, None, Op.is_gt)
        # negT = c0 + r01 * (big - c0)
        nc.vector.tensor_tensor(big[:], big[:], c0[:], Op.subtract)
        nc.vector.tensor_tensor(big[:], big[:], r01[:], Op.mult)
        negT = miscpool.tile([1, os_], FP32, tag="negT")
        nc.vector.tensor_tensor(negT[:], c0[:], big[:], Op.add)
        ones = miscpool.tile([1, 128], FP32, tag="ones")
        nc.vector.memset(ones[:], 1.0)

        # ---- weights: ternarize + transpose into Ws[ot][p, m, o] = w[o, 128m+p]
        # one tensor per o-tile so the first matmuls only wait on Ws[0]
        Ws = [
            wpool.tile([128, KC, NT], BF16, tag=f"W{ot}", name=f"W{ot}")
            for ot in range(NO)
        ]
        for j in range(os_ // 128):
            ot, jj = divmod(j, NT // 128)
            st = synpool.tile([128, in_], FP32, tag="st", name="st")
            nc.sync.dma_start(st[:], syn[ts(j, 128), :])
            ta = ternpool.tile([128, in_], BF16, tag="ta", name="ta")
            tb = ternpool.tile([128, in_], BF16, tag="tb", name="tb")
            nc.vector.tensor_scalar(ta[:], st[:], 1.0, None, Op.is_gt)
            nc.vector.tensor_scalar(tb[:], st[:], -1.0, None, Op.is_lt)
            nc.vector.tensor_tensor(ta[:], ta[:], tb[:], Op.subtract)
            nc.sync.dma_start(Ws[ot][:, :, ts(jj, 128)], ta[:], transpose=True)

        # ---- main sweep over batch tiles ----
        for bt in range(NB):
            sp = sppool.tile([128, in_], BF16, tag="sp", name="sp")
            nc.gpsimd.dma_start(sp[:], spike[ts(bt, 128), :])  # fp32->bf16 cast
            S = spool.tile([128, KC, 128], BF16, tag="S", name="S")
            nc.sync.dma_start(S[:], sp[:], transpose=True)
            pss = [pspool.tile([128, NT], FP32, tag="ps", name="ps") for _ in range(NO)]
            # m-outer / ot-inner: the stationary S[:, m, :] is reused across
            # o-tiles (PE reorder window pulls the single Ldweights ahead)
            for m in range(KC):
                for ot in range(NO):
                    nc.tensor.matmul(
                        pss[ot][:],
                        S[:, m, :],
                        Ws[ot][:, m, :],
                        start=(m == 0),
                        stop=False,
                    )
            for ot in range(NO):
                nc.tensor.matmul(
                    pss[ot][:], ones[:], negT[:, ts(ot, NT)], start=False, stop=True
                )
                ob = outpool.tile([128, NT], BF16, tag="ob", name="ob")
                nc.vector.tensor_scalar(ob[:], pss[ot][:], 0.0, None, Op.is_ge)
                nc.sync.dma_start(out[ts(bt, 128), ts(ot, NT)], ob[:])
    return out


def make_nc(bs=B // GB, os_=OUT // GO, in_=IN, repeat=1):
    from concourse import bacc
    from concourse.tile import TileContext

    nc = bacc.Bacc(trn_type="TRN2")
    with TileContext(nc) as tc:
        for r in range(repeat):
            build_core_program(nc, tc, bs, os_, in_, instance=r)
    nc.compile()
    return nc


_NC_CACHE = {}


def kernel(
    spike_input,
    synapse_states,
    membrane_potential,
    adaptive_threshold,
    refractory_count,
    _return_results=False,
):
    from concourse.bass_utils import run_bass_kernel_spmd

    spike_input = np.ascontiguousarray(np.asarray(spike_input, dtype=np.float32))
    synapse_states = np.ascontiguousarray(np.asarray(synapse_states, dtype=np.float32))
    membrane_potential = np.asarray(membrane_potential, dtype=np.float32)
    adaptive_threshold = np.asarray(adaptive_threshold, dtype=np.float32)
    refractory_count = np.asarray(refractory_count, dtype=np.float32)

    bs, os_ = B // GB, OUT // GO
    if "nc" not in _NC_CACHE:
        _NC_CACHE["nc"] = make_nc(bs, os_, IN)
    nc = _NC_CACHE["nc"]

    in_maps = []
    for c in range(GB * GO):
        bi, oj = divmod(c, GO)
        in_maps.append(
            {
                "spike": spike_input[bi * bs : (bi + 1) * bs],
                "syn": np.ascontiguousarray(
                    synapse_states[oj * os_ : (oj + 1) * os_]
                ),
                "thr": adaptive_threshold[None, oj * os_ : (oj + 1) * os_],
                "vmem": membrane_potential[None, oj * os_ : (oj + 1) * os_],
                "refrac": refractory_count[None, oj * os_ : (oj + 1) * os_],
            }
        )

    res = run_bass_kernel_spmd(nc, in_maps, core_ids=list(range(GB * GO)))

    full = np.empty((B, OUT), dtype=np.float32)
    for c in range(GB * GO):
        bi, oj = divmod(c, GO)
        full[bi * bs : (bi + 1) * bs, oj * os_ : (oj + 1) * os_] = res.results[c][
            "spikes"
        ].astype(np.float32)
    if _return_results:
        return full, res
    return full



# revision 8
# speedup vs baseline: 1.0482x; 1.0482x over previous
"""Trainium2 Bass kernel for LogicGatedSNN.

Math:
  w = ternarize(synapse_states)                  # {-1,0,1}
  current = spike_input @ w.T
  spikes[b,o] = (DECAY*vmem[o] + current*(1-refr) >= thr[o])

Implementation notes:
  * Weights are built as w2 = sign(x-1) + sign(x+1) in {-2,0,2} (two ACT
    sign ops + one DVE add), so the threshold is doubled: compare
    current2 >= T2 with T2 = 2*(thr - DECAY*vmem), or +-2e30 for
    refractory neurons (sign of DECAY*vmem - thr decides always/never).
  * Matmul runs in fp8e4 with perf_mode=DoubleRow: operands are sliced
    [:, 2*m2 : 2*m2+2, :] out of [128, KC, free] tiles, pairing two
    adjacent 128-row contraction chunks per instruction (the layout
    production tile_matmul uses).  spikes/w2 are exact in fp8e4, PSUM
    accumulates fp32 -> bit-exact vs the fp32 reference.
  * Epilogue: T2 is broadcast across partitions once via a K=1 matmul
    (ones^T @ T2) into PSUM -> SBUF [128, os_]; each output tile is a
    single DVE tensor_tensor is_ge (PSUM vs T2rep) -> fp8 {0,1}.
    No per-batch-tile bias matmuls.
  * DMA queues: syn loads + W transposes on the ACT HWDGE ring,
    spike transposes + stores on the SP ring, spike fp32->bf16 cast
    loads on SWDGE (gpsimd).

Sharding: 8 cores = 2 (batch) x 4 (out_features): per core
  spike [2048, 4096], syn [1024, 4096], out [2048, 1024].
"""

import sys

if "/opt/trn_rl_repo" not in sys.path:
    sys.path.insert(0, "/opt/trn_rl_repo")

import numpy as np

B, IN, OUT = 4096, 4096, 4096
GB, GO = 2, 4  # core grid: batch x out_features
DECAY = 0.8
_TENSORS = {}


def build_core_program(nc, tc, bs, os_, in_, instance=0):
    import concourse.mybir as mybir
    from concourse.bass import ts

    FP32 = mybir.dt.float32
    BF16 = mybir.dt.bfloat16
    FP8 = mybir.dt.float8e4
    Op = mybir.AluOpType
    Act = mybir.ActivationFunctionType
    DR = mybir.MatmulPerfMode.DoubleRow

    spike = nc.dram_tensor("spike", [bs, in_], FP32, kind="ExternalInput")
    syn = nc.dram_tensor("syn", [os_, in_], FP32, kind="ExternalInput")
    thr = nc.dram_tensor("thr", [1, os_], FP32, kind="ExternalInput")
    vmem = nc.dram_tensor("vmem", [1, os_], FP32, kind="ExternalInput")
    refrac = nc.dram_tensor("refrac", [1, os_], FP32, kind="ExternalInput")
    out = nc.dram_tensor("spikes", [bs, os_], FP8, kind="ExternalOutput")
    _TENSORS.update(spike=spike, syn=syn, thr=thr, vmem=vmem, refrac=refrac, out=out)

    KC = in_ // 128  # contraction chunks of 128
    KP = KC // 2  # DoubleRow chunk-pairs
    NB = bs // 128  # batch tiles
    NT = 512  # matmul free dim per o-tile
    NO = os_ // NT  # o-tiles
    NJ = os_ // 128  # weight row blocks

    with (
        tc.tile_pool(name="misc", bufs=1) as misc,
        tc.tile_pool(name="wst", bufs=2) as wst,
        tc.tile_pool(name="wsign", bufs=1) as wsign,
        tc.tile_pool(name="wtern", bufs=2) as wtern,
        tc.tile_pool(name="wtp", bufs=2) as wtp,
        tc.tile_pool(name="wf", bufs=1) as wf,
        tc.tile_pool(name="spp", bufs=2) as spp,
        tc.tile_pool(name="stp", bufs=2) as stp,
        tc.tile_pool(name="sfp", bufs=4) as sfp,
        tc.tile_pool(name="outp", bufs=4) as outp,
        tc.tile_pool(name="psp", bufs=4, space="PSUM") as psp,
        tc.tile_pool(name="psb", bufs=2, space="PSUM") as psb,
    ):
        # ---- effective doubled threshold T2[o], then broadcast to 128 rows
        tv = misc.tile([1, os_], FP32, tag="tv")
        vv = misc.tile([1, os_], FP32, tag="vv")
        rv = misc.tile([1, os_], FP32, tag="rv")
        nc.sync.dma_start(tv[:], thr[:, :])
        nc.sync.dma_start(vv[:], vmem[:, :])
        nc.sync.dma_start(rv[:], refrac[:, :])
        c0 = misc.tile([1, os_], FP32, tag="c0")
        nc.vector.tensor_scalar(c0[:], vv[:], DECAY, None, Op.mult)
        nc.vector.tensor_tensor(c0[:], c0[:], tv[:], Op.subtract)  # decay*v - thr
        big = misc.tile([1, os_], FP32, tag="big")
        nc.vector.tensor_scalar(big[:], c0[:], 0.0, None, Op.is_ge)
        nc.vector.tensor_scalar(big[:], big[:], -4e30, 2e30, Op.mult, Op.add)
        r01 = misc.tile([1, os_], FP32, tag="r01")
        nc.vector.tensor_scalar(r01[:], rv[:], 0.0, None, Op.is_gt)
        # T2 = -2*c0 + r01 * (big + 2*c0)
        t2 = misc.tile([1, os_], FP32, tag="t2")
        nc.vector.tensor_scalar(t2[:], c0[:], 2.0, None, Op.mult)
        nc.vector.tensor_tensor(big[:], big[:], t2[:], Op.add)
        nc.vector.tensor_tensor(big[:], big[:], r01[:], Op.mult)
        nc.vector.tensor_scalar(t2[:], c0[:], -2.0, None, Op.mult)
        nc.vector.tensor_tensor(t2[:], t2[:], big[:], Op.add)
        ones = misc.tile([1, 128], FP32, tag="ones")
        nc.vector.memset(ones[:], 1.0)
        bneg = misc.tile([128, 1], FP32, tag="bneg")
        bpos = misc.tile([128, 1], FP32, tag="bpos")
        nc.vector.memset(bneg[:], -1.0)
        nc.vector.memset(bpos[:], 1.0)
        t2rep = misc.tile([128, os_], FP32, tag="t2rep")
        for ot in range(NO):
            pb = psb.tile([128, NT], FP32, tag="pb", name="pb")
            nc.tensor.matmul(pb[:], ones[:], t2[:, ts(ot, NT)], start=True, stop=True)
            nc.scalar.copy(t2rep[:, ts(ot, NT)], pb[:])

        # ---- weights: sign(x-1)+sign(x+1) -> bf16 -> xbar transpose -> fp8
        # Wf[ot][p, m, o] = w2[o, 128m+p] (same chunk map as the S tiles)
        Wf = [wf.tile([128, KC, NT], FP8, tag=f"Wf{ot}", name=f"Wf{ot}") for ot in range(NO)]
        for j in range(NJ):
            ot, jj = divmod(j, NT // 128)
            st = wst.tile([128, in_], FP32, tag="st", name="st")
            nc.gpsimd.dma_start(st[:], syn[ts(j, 128), :])
            s1 = wsign.tile([128, in_], BF16, tag="s1", name="s1")
            s2 = wsign.tile([128, in_], BF16, tag="s2", name="s2")
            nc.scalar.activation(s1[:], st[:], Act.Sign, bias=bneg[:])
            nc.scalar.activation(s2[:], st[:], Act.Sign, bias=bpos[:])
            w2 = wtern.tile([128, in_], BF16, tag="w2", name="w2")
            nc.vector.tensor_tensor(w2[:], s1[:], s2[:], Op.add)
            wb = wtp.tile([128, KC, 128], BF16, tag="wb", name="wb")
            nc.scalar.dma_start_transpose(wb[:], w2[:])
            nc.vector.tensor_copy(Wf[ot][:, :, ts(jj, 128)], wb[:])

        # ---- main sweep over batch tiles ----
        for bt in range(NB):
            sp = spp.tile([128, in_], BF16, tag="sp", name="sp")
            nc.gpsimd.dma_start(sp[:], spike[ts(bt, 128), :])  # fp32->bf16 cast
            sb = stp.tile([128, KC, 128], BF16, tag="sb", name="sb")
            nc.sync.dma_start_transpose(sb[:], sp[:])
            sf = sfp.tile([128, KC, 128], FP8, tag="sf", name="sf")
            nc.vector.tensor_copy(sf[:], sb[:])
            pss = [psp.tile([128, NT], FP32, tag="ps", name="ps") for _ in range(NO)]
            for m2 in range(KP):
                for ot in range(NO):
                    nc.tensor.matmul(
                        pss[ot][:],
                        sf[:, 2 * m2 : 2 * m2 + 2, :],
                        Wf[ot][:, 2 * m2 : 2 * m2 + 2, :],
                        start=(m2 == 0),
                        stop=(m2 == KP - 1),
                        perf_mode=DR,
                    )
            for ot in range(NO):
                ob = outp.tile([128, NT], FP8, tag="ob", name="ob")
                nc.vector.tensor_tensor(
                    ob[:], pss[ot][:], t2rep[:, ts(ot, NT)], Op.is_ge
                )
                nc.sync.dma_start(out[ts(bt, 128), ts(ot, NT)], ob[:])
    return out


def make_nc(bs=B // GB, os_=OUT // GO, in_=IN):
    from concourse import bacc
    from concourse.tile import TileContext

    nc = bacc.Bacc(trn_type="TRN2")
    with TileContext(nc) as tc:
        build_core_program(nc, tc, bs, os_, in_)
    nc.compile()
    return nc


_NC_CACHE = {}


def kernel(
    spike_input,
    synapse_states,
    membrane_potential,
    adaptive_threshold,
    refractory_count,
    _return_results=False,
):
    from concourse.bass_utils import run_bass_kernel_spmd

    spike_input = np.ascontiguousarray(np.asarray(spike_input, dtype=np.float32))
    synapse_states = np.ascontiguousarray(np.asarray(synapse_states, dtype=np.float32))
    membrane_potential = np.asarray(membrane_potential, dtype=np.float32)
    adaptive_threshold = np.asarray(adaptive_threshold, dtype=np.float32)
    refractory_count = np.asarray(refractory_count, dtype=np.float32)

    bs, os_ = B // GB, OUT // GO
    if "nc" not in _NC_CACHE:
        _NC_CACHE["nc"] = make_nc(bs, os_, IN)
    nc = _NC_CACHE["nc"]

    in_maps = []
    for c in range(GB * GO):
        bi, oj = divmod(c, GO)
        in_maps.append(
            {
                "spike": spike_input[bi * bs : (bi + 1) * bs],
                "syn": np.ascontiguousarray(synapse_states[oj * os_ : (oj + 1) * os_]),
                "thr": adaptive_threshold[None, oj * os_ : (oj + 1) * os_],
                "vmem": membrane_potential[None, oj * os_ : (oj + 1) * os_],
                "refrac": refractory_count[None, oj * os_ : (oj + 1) * os_],
            }
        )

    res = run_bass_kernel_spmd(nc, in_maps, core_ids=list(range(GB * GO)))

    full = np.empty((B, OUT), dtype=np.float32)
    for c in range(GB * GO):
        bi, oj = divmod(c, GO)
        full[bi * bs : (bi + 1) * bs, oj * os_ : (oj + 1) * os_] = res.results[c][
            "spikes"
        ].astype(np.float32)
    if _return_results:
        return full, res
    return full
